# revision 5
# baseline (speedup 1.0000x reference)
"""CirLinear Trainium2 kernel v2: y = x @ build_weight(W, alphas, gumbels)^T + bias.

Strategy (8 NeuronCores, no collectives), 2x4 grid:
 - core c = tshard*4 + oshard: tokens [8192*tshard, +8192), out rows
   [512*oshard, +512)
 - circulant weight build per core (512 rows, two 256-row chunks) with:
   * host-pretiled weight input (contiguous wb load, no strided DMA)
   * bf16 accumulator (HWDGE scatter, DVE 2x tensor_tensor mode)
   * tree-based diagonal sums (bf16 2x) instead of 1x tensor_reduce
   * pad/scale stages on the scalar engine (ACT), frees DVE
 - matmul phases: A(tg0-3 x o0), A2(o1) use chunk0 only and overlap the
   chunk1 build; C1(o2), C2(o3) reuse the same x tiles; then token pairs
   P0..P5 x (o01, o23). 4 matmuls per LDWEIGHTS in quads, 2 in pairs.
 - x host-pretiled to [128, tg*ic*tok] bf16 so every x load is contiguous
 - DMA spread: x loads + even scatters/transposes on sync ring, odd ones on
   scalar ring, output stores on gpsimd (SWDGE)
"""
import sys

sys.path.insert(0, '/opt/trn_rl_repo')

import numpy as np

import concourse.bass as bass
from concourse import bacc
import concourse.mybir as mybir
from concourse.tile import TileContext
from concourse.bass_utils import run_bass_kernel_spmd

N_CORES = 8
T_SHARDS, O_SHARDS = 2, 4
BATCH, TOKENS, IN_F, OUT_F = 16, 1024, 2048, 2048
TOK_TOTAL = BATCH * TOKENS            # 16384
TOK = TOK_TOTAL // T_SHARDS           # 8192 tokens per core
ROWS = OUT_F // O_SHARDS              # 512 out-features per core
N_CH = 2                              # build chunks of 256 rows
SCALES = [2, 4, 8, 16, 32, 64]
N_IC = IN_F // 128                    # 16 contraction chunks
N_TG = TOK // 512                     # 16 token groups of 512
N_OS = ROWS // 128                    # 4 output-row subtiles

bf16 = mybir.dt.bfloat16
f32 = mybir.dt.float32
ADD = mybir.AluOpType.add
IDENT = mybir.ActivationFunctionType.Identity

_CACHE = {}


def sb(t, off, dims):
    return bass.AP(tensor=t.tensor, offset=off, ap=[list(t.ap[0])] + dims)


def _build_nc():
    nc = bacc.Bacc("TRN2", target_bir_lowering=False, debug=False, num_devices=N_CORES)
    x2 = nc.dram_tensor("x2", [128, N_TG * N_IC * 512], bf16, kind="ExternalInput")
    wsb = nc.dram_tensor("wsb", [128, N_CH * 4096], bf16, kind="ExternalInput")
    bias_t = nc.dram_tensor("bias_t", [128, N_OS], f32, kind="ExternalInput")
    alphas = nc.dram_tensor("alphas", [1, 7], f32, kind="ExternalInput")
    gumbels = nc.dram_tensor("gumbels", [1, 7], f32, kind="ExternalInput")
    out = nc.dram_tensor("out", [ROWS, TOK], f32, kind="ExternalOutput")

    w_locs = [nc.dram_tensor(f"w_loc{c}", [256, IN_F], bf16) for c in range(N_CH)]

    with TileContext(nc) as tc:
        # ---------- softmax(alphas + gumbels) broadcast to 128 partitions ----------
        asb = nc.alloc_sbuf_tensor("asb", [128, 7], f32).ap()
        gsb = nc.alloc_sbuf_tensor("gsb", [128, 7], f32).ap()
        a_bc = nc.alloc_sbuf_tensor("a_bc", [128, 7], f32).ap()
        a_div = nc.alloc_sbuf_tensor("a_div", [128, 7], f32).ap()
        ssum = nc.alloc_sbuf_tensor("ssum", [128, 1], f32).ap()
        nc.gpsimd.dma_start(out=asb, in_=bass.AP(tensor=alphas, offset=0, ap=[[0, 128], [1, 7]]))
        nc.gpsimd.dma_start(out=gsb, in_=bass.AP(tensor=gumbels, offset=0, ap=[[0, 128], [1, 7]]))
        nc.vector.tensor_tensor(out=asb, in0=asb, in1=gsb, op=ADD)
        nc.scalar.activation(out=asb, in_=asb, func=mybir.ActivationFunctionType.Exp)
        nc.vector.tensor_reduce(out=ssum, in_=asb, axis=mybir.AxisListType.X, op=ADD)
        nc.vector.reciprocal(out=ssum, in_=ssum)
        nc.vector.tensor_scalar_mul(a_bc, asb, ssum)
        for idx, b in enumerate(SCALES, start=1):
            nc.vector.tensor_scalar_mul(a_div[:, idx:idx + 1], a_bc[:, idx:idx + 1], 1.0 / b)

        # ---------- bias: host-transposed [128, 4] ----------
        bias_sb = nc.alloc_sbuf_tensor("bias_sb", [128, N_OS], f32).ap()
        nc.gpsimd.dma_start(out=bias_sb, in_=bias_t.ap())

        # ---------- build buffers ----------
        wb = nc.alloc_sbuf_tensor("wb", [128, 4096], bf16).ap()
        acc2 = [nc.alloc_sbuf_tensor(f"acc{c}", [128, 4096], bf16).ap() for c in range(2)]
        tr2 = [nc.alloc_sbuf_tensor(f"tr{i}", [128, 4096], bf16).ap() for i in range(2)]
        dpad2 = [nc.alloc_sbuf_tensor(f"dpad{i}", [128, 4096], bf16).ap() for i in range(2)]
        wTs = [nc.alloc_sbuf_tensor(f"wT{c}", [128, N_IC * 256], bf16).ap()
               for c in range(N_CH)]

        def build_chunk(ch):
            acc = acc2[ch]
            nc.vector.tensor_scalar_mul(acc, wb, a_bc[:, 0:1])
            for idx, b in enumerate(SCALES, start=1):
                nv = 64 // b
                tr = tr2[idx % 2]
                dpad = dpad2[idx % 2]
                # diagonal sums via shifted pair-adds, no padding:
                #   S_l[u,blk,k,c] = S_{l-1}[u,blk,k,c] + S_{l-1}[u,blk,k+h,(c+h)%b]
                # wraparound split into two contiguous c-range instructions;
                # c (stride 1) stays innermost for the DVE 2x bf16 mode
                # packed intermediate layout [u][k][blk][c] lets (k,blk) merge
                # into one stride-b dim, keeping every AP at <=3 free dims
                nlev = b.bit_length() - 1        # log2(b)
                off_out = 0
                off_in = 0
                for lev in range(1, nlev + 1):
                    h = b >> lev
                    if lev == 1:
                        u_st, k_st, src, base = b * 64, 64, wb, 0
                    else:
                        u_st, k_st, src, base = 2 * h * nv * b, nv * b, tr, off_in

                    def dims(ck):
                        return [[u_st, nv], [b, h * nv], [1, ck]]

                    def odims(ck):
                        return [[b, nv * nv * h], [1, ck]]

                    # c in [0, b-h): partner at (k+h, c+h)
                    nc.vector.tensor_tensor(
                        out=sb(tr, off_out, odims(b - h)),
                        in0=sb(src, base, dims(b - h)),
                        in1=sb(src, base + h * k_st + h, dims(b - h)), op=ADD)
                    # c in [b-h, b): partner at (k+h, c+h-b)
                    nc.vector.tensor_tensor(
                        out=sb(tr, off_out + (b - h), odims(h)),
                        in0=sb(src, base + (b - h), dims(h)),
                        in1=sb(src, base + h * k_st, dims(h)), op=ADD)
                    off_in = off_out
                    off_out = off_in + (4096 >> lev)
                off_final = off_in
                # dpad = diag sums * (a_b / b), duplicated 2x for wraparound (DVE)
                dsrc = sb(tr, off_final, [[64, nv], [b, nv], [1, b]])
                for half in range(2):
                    ddst = sb(dpad, half * b, [[128, nv], [2 * b, nv], [1, b]])
                    nc.vector.tensor_scalar_mul(ddst, dsrc, a_div[:, idx:idx + 1])
                # expand: acc[u,blk,r,j] += dpad[u, blk, b - r + j]
                # (split over the smaller of u/r to stay at 3 free dims)
                if nv <= b:
                    for u in range(nv):
                        aout = sb(acc, u * b * 64, [[b, nv], [64, b], [1, b]])
                        din = sb(dpad, u * 128 + b, [[2 * b, nv], [-1, b], [1, b]])
                        nc.vector.tensor_tensor(out=aout, in0=aout, in1=din, op=ADD)
                else:
                    for r in range(b):
                        aout = sb(acc, r * 64, [[b * 64, nv], [b, nv], [1, b]])
                        din = sb(dpad, b - r, [[128, nv], [2 * b, nv], [1, b]])
                        nc.vector.tensor_tensor(out=aout, in0=aout, in1=din, op=ADD)

        def scatter_transpose(ch):
            acc = acc2[ch]
            wl = w_locs[ch]
            wl4 = wl.ap().rearrange("(q r) (p s) -> q p r s", r=64, s=64)
            for q in range(4):
                eng = nc.gpsimd if q < 2 else nc.sync
                eng.dma_start(out=wl4[q], in_=acc[q * 32:(q + 1) * 32, :])
            for ic in range(N_IC):
                nc.sync.dma_start(out=wTs[ch][:, ic * 256:(ic + 1) * 256],
                                  in_=wl.ap()[:, ic * 128:(ic + 1) * 128],
                                  transpose=True)

        with (
            tc.tile_pool(name="xt", bufs=12) as xt_pool,
            tc.tile_pool(name="psum", bufs=8, space="PSUM") as psum_pool,
            tc.tile_pool(name="osb", bufs=3) as osb_pool,
        ):
            xts = {}

            def load_xt(tg, engs=(nc.sync, nc.sync)):
                lo = xt_pool.tile([128, 4096], bf16, name=f"xl{tg}", tag="xt")
                hi = xt_pool.tile([128, 4096], bf16, name=f"xh{tg}", tag="xt")
                engs[0].dma_start(out=lo[:], in_=x2.ap()[:, tg * 8192: tg * 8192 + 4096])
                engs[1].dma_start(out=hi[:], in_=x2.ap()[:, tg * 8192 + 4096: (tg + 1) * 8192])
                xts[tg] = (lo, hi)

            def mm_group(tgs, os_):
                pss = {}
                for o in os_:
                    for tg in tgs:
                        pss[(o, tg)] = psum_pool.tile([128, 512], f32, name="ps", tag="ps")
                for ic in range(N_IC):
                    for o in os_:
                        w_ap = wTs[o // 2][:, ic * 256 + (o % 2) * 128:
                                           ic * 256 + (o % 2) * 128 + 128]
                        for tg in tgs:
                            xt = xts[tg][0] if ic < 8 else xts[tg][1]
                            rhs = xt[:, (ic % 8) * 512: (ic % 8 + 1) * 512]
                            nc.tensor.matmul(pss[(o, tg)][:], w_ap, rhs,
                                             start=(ic == 0), stop=(ic == N_IC - 1))
                for o in os_:
                    for i in range(0, len(tgs), 2):
                        ta, tb = tgs[i], tgs[i + 1]
                        ot = osb_pool.tile([128, 1024], f32, name="ot", tag="ot")
                        nc.scalar.activation(out=ot[:, 0:512], in_=pss[(o, ta)][:],
                                             func=IDENT, bias=bias_sb[:, o:o + 1], scale=1.0)
                        nc.scalar.activation(out=ot[:, 512:1024], in_=pss[(o, tb)][:],
                                             func=IDENT, bias=bias_sb[:, o:o + 1], scale=1.0)
                        nc.gpsimd.dma_start(
                            out=out.ap()[o * 128:(o + 1) * 128, ta * 512: ta * 512 + 1024],
                            in_=ot[:])

            # ---------- emission order ----------
            nc.sync.dma_start(out=wb, in_=wsb.ap()[:, 0:4096])          # wb chunk0
            for tg in range(4):                                          # T0 prefetch
                load_xt(tg)
            build_chunk(0)
            # wb chunk1 load (waits for chunk0 tree-L1 WAR on wb)
            nc.sync.dma_start(out=wb, in_=wsb.ap()[:, 4096:8192])
            scatter_transpose(0)
            load_xt(4)
            load_xt(5)
            build_chunk(1)
            scatter_transpose(1)
            for tg in range(6, N_TG):
                load_xt(tg)
            # matmul phases: A (chunk0), P0h0 (chunk0) covers chunk1 build,
            # then C (chunk1 rows on the still-resident T0 x tiles), then pairs
            mm_group([0, 1, 2, 3], [0])
            mm_group([0, 1, 2, 3], [1])
            mm_group([4, 5], [0, 1])
            mm_group([0, 1, 2, 3], [2])
            mm_group([0, 1, 2, 3], [3])
            mm_group([4, 5], [2, 3])
            for pi in range(1, 6):
                ta = 4 + 2 * pi
                mm_group([ta, ta + 1], [0, 1])
                mm_group([ta, ta + 1], [2, 3])

    nc.compile()
    return nc


def make_in_maps(x, weight, bias, alphas, gumbels):
    import ml_dtypes
    bf = ml_dtypes.bfloat16
    xf = np.asarray(x, np.float32).reshape(T_SHARDS, TOK, IN_F)
    xslices = []
    for ts in range(T_SHARDS):
        a = xf[ts].reshape(N_TG, 512, N_IC, 128).transpose(3, 0, 2, 1)
        xslices.append(np.ascontiguousarray(a.reshape(128, N_TG * N_IC * 512)).astype(bf))
    weight = np.asarray(weight, np.float32)
    bias = np.asarray(bias, np.float32)
    wslices, bslices = [], []
    for o in range(O_SHARDS):
        ws = weight[o * ROWS:(o + 1) * ROWS]                 # [512, 2048]
        w4 = ws.reshape(N_CH, 4, 64, 32, 64).transpose(1, 3, 0, 2, 4)
        wslices.append(np.ascontiguousarray(w4.reshape(128, N_CH * 4096)).astype(bf))
        bslices.append(np.ascontiguousarray(
            bias[o * ROWS:(o + 1) * ROWS].reshape(N_OS, 128).T).astype(np.float32))
    al = np.asarray(alphas, np.float32).reshape(1, 7)
    gu = np.asarray(gumbels, np.float32).reshape(1, 7)
    in_maps = []
    for c in range(N_CORES):
        t, o = divmod(c, O_SHARDS)
        in_maps.append({"x2": xslices[t], "wsb": wslices[o], "bias_t": bslices[o],
                        "alphas": al, "gumbels": gu})
    return in_maps


def kernel(x, weight, bias, alphas, gumbels):
    if "nc" not in _CACHE:
        _CACHE["nc"] = _build_nc()
    nc = _CACHE["nc"]
    in_maps = make_in_maps(x, weight, bias, alphas, gumbels)
    res = run_bass_kernel_spmd(nc, in_maps, core_ids=list(range(N_CORES)))
    row_blocks = []
    for o in range(O_SHARDS):
        row_blocks.append(np.concatenate(
            [res.results[t * O_SHARDS + o]["out"] for t in range(T_SHARDS)], axis=1))
    full_t = np.concatenate(row_blocks, axis=0)              # [2048, 16384]
    return np.ascontiguousarray(full_t.T).reshape(BATCH, TOKENS, OUT_F)
